# revision 1
# baseline (speedup 1.0000x reference)
"""Trainium2 Bass kernel for nn_CESAR_24309514895978 (ragged_sequence).

Math (per batch b):
  m0 = (attention_masks==1)&(token_type_ids==0); m1 = (attention_masks==1)&(token_type_ids==1)
  score[i,j] = |emb_n[i] . emb_n[j]|   (L2-normalized embeddings)
  logits[i,j] = (emb@Wq.T+bq)[i] . (emb@Wk.T+bk)[j]
  cs[b] = sum_{valid ij} softmax_flat(logits | pair_mask)[i,j] * score[i,j]

Constant folding (host, once): the projections only enter through
  logits = embaug @ A_aug @ embaug.T,  embaug = [emb, 1],
  A_aug = [[Wq.T@Wk, Wq.T@bk], [bq.T@Wk, bq.bk]]   ((D+1)x(D+1))
so the per-batch device work is two chained matmuls instead of three.

Device, per batch (data-parallel: 2 batches per core x 8 cores, fp32r matmuls):
  - rsq[j] = sum_d emb[j,d]^2 (DVE squares+adds, one ones-column matmul);
    r = 1/sqrt (ACT Sqrt + DVE reciprocal); W2 = r row via gpsimd broadcast
  - P = A_aug @ embaug.T   (stage 1, 8 PSUM banks, db-outer accumulation;
    the u-row rides the PSUM->SBUF copy bias, c0 rides the prow copy bias)
  - L = embaug.T.T @ P + one K=3 matmul adding the -1e30 ragged-pair masks
    and the rank-1 prow term (rows: m0neg/ones/ones x ones/m1neg/prow)
  - M = masked max (DVE reduces + gpsimd partition_all_reduce)
  - E = exp(L - M) on ACT with accum_out -> Z partial sums
  - W partials = sum_j E * |G| * r_j  (G = gram matmul; |.| on ACT; stt fused)
Host: r_i scaling + final sums + W/Z division (tiny) + input layout/rounding.
"""
import numpy as np

import concourse.bass_isa as bass_isa
import concourse.tile as tile
from concourse import bacc, mybir
from concourse.bass_utils import run_bass_kernel_spmd

B, S, D = 16, 512, 1024
NCORES = 8
BPC = B // NCORES          # batches per core
NCH = D // 128             # 8 contraction chunks
NIC = S // 128             # 4 i-chunks
DA = D + 1                 # augmented dim
NEG = np.float32(-1e30)

F32 = mybir.dt.float32
F32R = mybir.dt.float32r
AFT = mybir.ActivationFunctionType
ALU = mybir.AluOpType
AX = mybir.AxisListType

PROFILE = False            # set True (e.g. from test.py) to capture NTFF profile
LAST_RESULTS = None        # BassKernelResults of the last run (for test.py)

_built = None


def _to_fp32r(x: np.ndarray) -> np.ndarray:
    """Round fp32 -> fp32r encoding (RNE to 11 explicit mantissa bits)."""
    u = np.ascontiguousarray(x, dtype=np.float32).view(np.uint32).astype(np.uint64)
    u = (u + 0x7FF + ((u >> 12) & 1)) & np.uint64(0xFFFFF000)
    return u.astype(np.uint32).view(np.float32)


def _build():
    global _built
    if _built is not None:
        return _built

    nc = bacc.Bacc("TRN2", target_bir_lowering=False, debug=False)

    embT_d = nc.dram_tensor("embT", [BPC, NCH, 128, S], F32R, kind="ExternalInput").ap()
    # AT[db, da] = A_aug[da, db]; rows 0..1023 in 8 chunks + row 1024 separate
    at_d = nc.dram_tensor("at", [DA, DA], F32R, kind="ExternalInput").ap()
    lrows_d = nc.dram_tensor("lrows", [BPC, 3, S], F32R, kind="ExternalInput").ap()
    rrows_d = nc.dram_tensor("rrows", [BPC, 2, S], F32R, kind="ExternalInput").ap()
    ucol_d = nc.dram_tensor("ucol", [128, NCH], F32, kind="ExternalInput").ap()
    c0_d = nc.dram_tensor("c0", [1, 1], F32, kind="ExternalInput").ap()
    ones_d = nc.dram_tensor("ones", [128, 1], F32R, kind="ExternalInput").ap()
    onesrow_d = nc.dram_tensor("onesrow", [1, S], F32R, kind="ExternalInput").ap()

    zw_d = nc.dram_tensor("zw", [BPC, 2, 128, NIC], F32, kind="ExternalOutput").ap()
    rout_d = nc.dram_tensor("rout", [BPC, S], F32, kind="ExternalOutput").ap()

    with tile.TileContext(nc) as tc:
        with (
            tc.tile_pool(name="apool", bufs=9) as apool,
            tc.tile_pool(name="spool", bufs=1) as spool,
            tc.tile_pool(name="epool", bufs=16) as epool,
            tc.tile_pool(name="sqpool", bufs=3) as sqpool,
            tc.tile_pool(name="paugpool", bufs=18) as paugpool,
            tc.tile_pool(name="w2pool", bufs=2) as w2pool,
            tc.tile_pool(name="gapool", bufs=2) as gapool,
            tc.tile_pool(name="gwpool", bufs=4) as gwpool,
            tc.tile_pool(name="Epool", bufs=2) as Epool,
            tc.tile_pool(name="scrpool", bufs=1) as scrpool,
            tc.tile_pool(name="tiny", bufs=2) as tiny,
            tc.tile_pool(name="lrpool", bufs=2) as lrpool,
            tc.tile_pool(name="ps", bufs=8, space="PSUM") as ps,
        ):
            # ---- first chunk pair goes absolutely first (PE start gate),
            # then the tiny loads, then the remaining big chunks interleaved.
            emb_all = [[None] * NCH for _ in range(BPC)]
            at_t = []
            t = apool.tile([128, DA], F32R, tag="a", name="at_0")
            nc.sync.dma_start(out=t[:], in_=at_d[0:128, :])
            at_t.append(t)
            t = epool.tile([128, S], F32R, tag="emb", name="emb0_0")
            nc.sync.dma_start(out=t[:], in_=embT_d[0, 0])
            emb_all[0][0] = t

            ones_col = spool.tile([128, 1], F32R, tag="ones_col")
            nc.sync.dma_start(out=ones_col[:], in_=ones_d)
            onesrow_t = spool.tile([1, S], F32R, tag="onesrow")
            nc.sync.dma_start(out=onesrow_t[:], in_=onesrow_d)
            ucol_t = spool.tile([128, NCH], F32, tag="ucol")
            nc.sync.dma_start(out=ucol_t[:], in_=ucol_d)
            c0_t = spool.tile([1, 1], F32, tag="c0")
            nc.sync.dma_start(out=c0_t[:], in_=c0_d)
            lr_all = []
            for b in range(BPC):
                lr_t = lrpool.tile([3, S], F32R, tag="lr", name=f"lr{b}")
                nc.sync.dma_start(out=lr_t[:], in_=lrows_d[b])
                lr_all.append(lr_t)

            for c in range(1, NCH):
                t = epool.tile([128, S], F32R, tag="emb", name=f"emb0_{c}")
                nc.sync.dma_start(out=t[:], in_=embT_d[0, c])
                emb_all[0][c] = t
                t = apool.tile([128, DA], F32R, tag="a", name=f"at_{c}")
                nc.sync.dma_start(out=t[:], in_=at_d[c * 128 : (c + 1) * 128, :])
                at_t.append(t)

            for b in range(BPC):
                # ---- load this batch's emb
                if b > 0:
                    for c in range(NCH):
                        t = epool.tile([128, S], F32R, tag="emb", name=f"emb{b}_{c}")
                        nc.sync.dma_start(out=t[:], in_=embT_d[b, c])
                        emb_all[b][c] = t
                emb_t = emb_all[b]
                lr_t = lr_all[b]

                # ---- stage 1: P = A_aug @ embaug.T  (db-outer over 8 banks);
                # the ones-row term (u) is folded into the copy bias below.
                st1 = [ps.tile([128, S], F32, tag="ps", name=f"st1_{b}_{da}")
                       for da in range(NCH)]
                prow_ps = ps.tile([1, S], F32, tag="ps")
                for db in range(NCH):
                    for da in range(NCH):
                        nc.tensor.matmul(st1[da][:],
                                         at_t[db][:, da * 128 : (da + 1) * 128],
                                         emb_t[db][:],
                                         start=(db == 0), stop=(db == NCH - 1))
                    # prow (P row 1024) rides the same chunk: 9 MMs per chunk
                    # pair matches the DMA arrival rate for batch 0
                    nc.tensor.matmul(prow_ps[:], at_t[db][:, D : D + 1],
                                     emb_t[db][:],
                                     start=(db == 0), stop=(db == NCH - 1))
                paug = []
                for da in range(NCH):
                    pt = paugpool.tile([128, S], F32R, tag="paug")
                    if da % 2 == 0:
                        nc.scalar.activation(out=pt[:], in_=st1[da][:],
                                             func=AFT.Identity,
                                             bias=ucol_t[:, da : da + 1], scale=1.0)
                    else:
                        nc.vector.tensor_scalar_add(pt[:], st1[da][:],
                                                    ucol_t[:, da : da + 1])
                    paug.append(pt)
                # P row 1024 (the bq-side rank-1 term); c0 folded into the bias
                prow = tiny.tile([1, S], F32R, tag="prow")
                nc.scalar.activation(out=prow[:], in_=prow_ps[:],
                                     func=AFT.Identity, bias=c0_t[:], scale=1.0)
                # rhs rows for the combined mask+prow matmul (K=3):
                # p0 = ones, p1 = m1neg (host), p2 = prow (device)
                rr3 = lrpool.tile([3, S], F32R, tag="rr3")
                nc.sync.dma_start(out=rr3[0:2, :], in_=rrows_d[b])
                nc.sync.dma_start(out=rr3[2:3, :], in_=prow[:])

                # ---- rsq / r / W2
                sqacc = sqpool.tile([128, S], F32R, tag="sqacc", bufs=2)
                sq0 = sqpool.tile([128, S], F32, tag="sq")
                nc.vector.tensor_mul(sq0[:], emb_t[0][:].bitcast(F32),
                                     emb_t[0][:].bitcast(F32))
                for c in range(1, NCH):
                    sq = sqpool.tile([128, S], F32, tag="sq")
                    nc.vector.tensor_mul(sq[:], emb_t[c][:].bitcast(F32),
                                         emb_t[c][:].bitcast(F32))
                    if c < NCH - 1:
                        nc.vector.tensor_add(sq0[:], sq0[:], sq[:])
                    else:
                        nc.vector.tensor_add(sqacc[:], sq0[:], sq[:])
                rsq_ps = ps.tile([1, S], F32, tag="ps")
                nc.tensor.matmul(rsq_ps[:], ones_col[:], sqacc[:],
                                 start=True, stop=True)
                s_row = tiny.tile([1, S], F32, tag="srow")
                nc.scalar.activation(out=s_row[:], in_=rsq_ps[:], func=AFT.Sqrt,
                                     bias=0.0, scale=1.0)
                r_row = tiny.tile([1, S], F32, tag="rrow")
                nc.vector.reciprocal(out=r_row[:], in_=s_row[:])
                nc.sync.dma_start(out=rout_d[b], in_=r_row[:])
                W2 = w2pool.tile([128, S], F32, tag="w2")
                nc.gpsimd.partition_broadcast(W2[:], r_row[0:1, :], channels=128)

                # ---- stage 2: L chunks + masks; per-chunk max
                mx = tiny.tile([128, NIC], F32, tag="mx")
                L_ps = []
                for ic in range(NIC):
                    Lp = ps.tile([128, S], F32, tag="ps", name=f"L_{b}_{ic}")
                    for da in range(NCH):
                        nc.tensor.matmul(Lp[:], emb_t[da][:, ic * 128 : (ic + 1) * 128],
                                         paug[da][:], start=(da == 0), stop=False)
                    nc.tensor.matmul(Lp[:], lr_t[:, ic * 128 : (ic + 1) * 128],
                                     rr3[:], start=False, stop=True)
                    nc.vector.reduce_max(mx[:, ic : ic + 1], Lp[:], axis=AX.X)
                    L_ps.append(Lp)

                # ---- global masked max -> -M in [128,1]
                par = tiny.tile([128, NIC], F32, tag="par")
                nc.gpsimd.partition_all_reduce(par[:], mx[:], channels=128,
                                               reduce_op=bass_isa.ReduceOp.max)
                negm128 = tiny.tile([128, 1], F32, tag="negm128")
                nc.vector.reduce_max(negm128[:], par[:], axis=AX.X, negate=True)

                # ---- gram chunks -> Gw = |G| * r_j
                gw_t = []
                for ic in range(NIC):
                    Gp = ps.tile([128, S], F32, tag="ps", name=f"G_{b}_{ic}")
                    for c in range(NCH):
                        nc.tensor.matmul(Gp[:], emb_t[c][:, ic * 128 : (ic + 1) * 128],
                                         emb_t[c][:], start=(c == 0), stop=(c == NCH - 1))
                    ga = gapool.tile([128, S], F32, tag="ga")
                    nc.scalar.activation(out=ga[:], in_=Gp[:], func=AFT.Abs,
                                         bias=0.0, scale=1.0)
                    gw = gwpool.tile([128, S], F32, tag="gw")
                    nc.vector.tensor_mul(gw[:], ga[:], W2[:])
                    gw_t.append(gw)

                # ---- exp + fused weighted reductions
                zwcols = tiny.tile([128, 2 * NIC], F32, tag="zwc")
                zcols = zwcols[:, 0:NIC]
                wcols = zwcols[:, NIC : 2 * NIC]
                for ic in range(NIC):
                    E = Epool.tile([128, S], F32, tag="E")
                    nc.scalar.activation(out=E[:], in_=L_ps[ic][:], func=AFT.Exp,
                                         bias=negm128[:], scale=1.0,
                                         accum_out=zcols[:, ic : ic + 1])
                    scr = scrpool.tile([128, S], F32, tag="scr")
                    nc.vector.scalar_tensor_tensor(
                        out=scr[:], in0=gw_t[ic][:], scalar=1.0, in1=E[:],
                        op0=ALU.mult, op1=ALU.mult,
                        accum_out=wcols[:, ic : ic + 1])

                nc.sync.dma_start(out=zw_d[b, 0], in_=zcols[:])
                nc.sync.dma_start(out=zw_d[b, 1], in_=wcols[:])

    nc.compile()
    _built = nc
    return nc


def kernel(embeddings, Wq, bq, Wk, bk, attention_masks, token_type_ids):
    global LAST_RESULTS
    nc = _build()

    embeddings = np.ascontiguousarray(np.asarray(embeddings, dtype=np.float32))
    Wq = np.asarray(Wq, dtype=np.float32)
    Wk = np.asarray(Wk, dtype=np.float32)
    bq = np.asarray(bq, dtype=np.float32)
    bk = np.asarray(bk, dtype=np.float32)
    am = np.asarray(attention_masks)
    tt = np.asarray(token_type_ids)

    # host-side layout + constant folding + fp32r rounding
    embT = _to_fp32r(embeddings.transpose(0, 2, 1)).reshape(B, NCH, 128, S)

    Wq64, Wk64 = Wq.astype(np.float64), Wk.astype(np.float64)
    A_aug = np.empty((DA, DA), np.float64)
    A_aug[:D, :D] = Wq64.T @ Wk64                  # A[d,d'] = sum_e Wq[e,d] Wk[e,d']
    A_aug[:D, D] = Wq64.T @ bk.astype(np.float64)   # u
    A_aug[D, :D] = Wk64.T @ bq.astype(np.float64)   # v
    A_aug[D, D] = float(bq.astype(np.float64) @ bk.astype(np.float64))
    AT = _to_fp32r(np.ascontiguousarray(A_aug.T).astype(np.float32))

    tok = am == 1
    m0 = tok & (tt == 0)
    m1 = tok & (tt == 1)
    m0neg = np.where(m0, np.float32(0.0), NEG).astype(np.float32)
    m1neg = np.where(m1, np.float32(0.0), NEG).astype(np.float32)
    ones_row = np.ones((B, 1, S), np.float32)
    lrows = _to_fp32r(np.concatenate([m0neg[:, None, :], ones_row, ones_row], axis=1))
    rrows = _to_fp32r(np.concatenate([ones_row, m1neg[:, None, :]], axis=1))
    ucol = np.ascontiguousarray(
        A_aug[:D, D].astype(np.float32).reshape(NCH, 128).T)        # [128, NCH]
    c0 = np.array([[A_aug[D, D]]], np.float32)

    in_maps = []
    for i in range(NCORES):
        sl = slice(i * BPC, (i + 1) * BPC)
        in_maps.append({
            "embT": np.ascontiguousarray(embT[sl]),
            "at": AT,
            "lrows": np.ascontiguousarray(lrows[sl]),
            "rrows": np.ascontiguousarray(rrows[sl]),
            "ones": np.ones((128, 1), np.float32),
            "onesrow": np.ones((1, S), np.float32),
            "ucol": ucol, "c0": c0,
        })

    res = run_bass_kernel_spmd(nc, in_maps, core_ids=list(range(NCORES)),
                               trace=PROFILE)
    LAST_RESULTS = res

    valid = m0.any(axis=1) & m1.any(axis=1)
    cs = np.zeros(B, np.float64)
    for i in range(NCORES):
        for j in range(BPC):
            b = i * BPC + j
            if not valid[b]:
                continue
            zcols = res.results[i]["zw"][j, 0].astype(np.float64)   # [128, NIC]
            wcols = res.results[i]["zw"][j, 1].astype(np.float64)
            r = res.results[i]["rout"][j].astype(np.float64)        # [S]
            ri = r.reshape(NIC, 128).T                              # [128, NIC]
            z = zcols.sum()
            w = (wcols * ri).sum()
            cs[b] = w / (z + 1e-30)
    return cs.astype(np.float32)



# revision 2
# speedup vs baseline: 2.3460x; 2.3460x over previous
"""Trainium2 Bass kernel for nn_CESAR_24309514895978 (ragged_sequence).

Math (per batch b):
  m0 = (am==1)&(tt==0); m1 = (am==1)&(tt==1)
  score[i,j] = |emb_n[i] . emb_n[j]|   (L2-normalized embeddings)
  logits[i,j] = (emb@Wq.T+bq)[i] . (emb@Wk.T+bk)[j]
  cs[b] = sum_{valid ij} softmax_flat(logits | i in m0, j in m1)[i,j] * score[i,j]

Ragged compaction: only ~25% of tokens are in m0 and ~25% in m1, so the
host gathers the valid tokens and the device works on compacted panels:
rows = m1 tokens of 2 batches packed (<=2*128), cols = m0 tokens (free
dim ~260).  Matmul cost scales with the free dim only, so rows use the
partition dim (2 chunks) and cols the free dim.

Constant folding (host): logits = embaug_r @ A_aug @ embaug_c.T with
A_aug = [[Wk.T@Wq, Wk.T@bq], [bk.T@Wq, bq.bk]].  The device gets
  at  = (Wq.T@Wk)[db, da]  (stage-1 lhsT, bf16 to halve its DMA)
  ucol= Wk.T@bq            (bias riding the PSUM->SBUF copy of P)
  prow= emb_c@(Wq.T@bk)+bq.bk  (host-computed rank-1 row, in the mask mm)
Norms r=1/||emb|| are computed on the host; r_c rides a broadcast row,
r_r is applied host-side to the W partials.

Batch identity inside a packed panel is enforced with a K=4 mask matmul
(sum of non-positive rank-1 terms; no large-value cancellation):
  [ones, b0r, b1r, padr] x [prow, -1e30*b1c, -1e30*b0c, -1e30*ones]
No on-device max: exp uses a constant bias -M0 (uploaded, so a retry
with a larger M0 needs no recompile); W/Z ratio cancels the shift.

Device per core: stage1 P = at.T @ embc (64 mm), gram G = embr.T @ embc
(16 mm), stage2 L = embr.T @ paug + mask (18 mm); exp+accum -> Z rows,
stt(gw,E)+accum -> W rows.  Host: segment-sum rows by batch, cs = W/Z.
"""
import numpy as np
import ml_dtypes

import concourse.tile as tile
from concourse import bacc, mybir
from concourse.bass_utils import run_bass_kernel_spmd

B, S, D = 16, 512, 1024
NCORES = 8
NCH = D // 128             # 8 contraction chunks
NEG = np.float32(-1e30)
M0 = 60.0                  # logit shift; exp(L - M0), max logit ~58
EPS = 1e-12

F32 = mybir.dt.float32
F32R = mybir.dt.float32r
BF16 = mybir.dt.bfloat16
AFT = mybir.ActivationFunctionType
ALU = mybir.AluOpType

PROFILE = False            # set True (e.g. from test.py) to capture NTFF profile
LAST_RESULTS = None        # BassKernelResults of the last run (for test.py)

ST2_F32R = False           # stage2 (embr x paug) in f32r instead of bf16

_builds = {}


def _to_fp32r(x: np.ndarray) -> np.ndarray:
    """Round fp32 -> fp32r encoding (RNE to 11 explicit mantissa bits)."""
    u = np.ascontiguousarray(x, dtype=np.float32).view(np.uint32).astype(np.uint64)
    u = (u + 0x7FF + ((u >> 12) & 1)) & np.uint64(0xFFFFF000)
    return u.astype(np.uint32).view(np.float32)


def _bf16(x: np.ndarray) -> np.ndarray:
    return np.ascontiguousarray(np.asarray(x, np.float32)).astype(ml_dtypes.bfloat16)


def _build(nr: int, c: int, st2_f32r: bool):
    key = (nr, c, st2_f32r)
    if key in _builds:
        return _builds[key]

    R = nr * 128
    nc = bacc.Bacc("TRN2", target_bir_lowering=False, debug=False)

    at_d = nc.dram_tensor("at", [NCH, 128, D], BF16, kind="ExternalInput").ap()
    embc_d = nc.dram_tensor("embc", [NCH, 128, c], BF16, kind="ExternalInput").ap()
    embr_d = nc.dram_tensor("embr", [NCH, 128, R], BF16, kind="ExternalInput").ap()
    if st2_f32r:
        embr32_d = nc.dram_tensor("embr32", [NCH, 128, R], F32R,
                                  kind="ExternalInput").ap()
    mrows_d = nc.dram_tensor("mrows", [4, R], F32R, kind="ExternalInput").ap()
    rhs4_d = nc.dram_tensor("rhs4", [4, c], F32R, kind="ExternalInput").ap()
    ucol_d = nc.dram_tensor("ucol", [128, NCH], F32, kind="ExternalInput").ap()
    rrow_d = nc.dram_tensor("rrow", [1, c], F32, kind="ExternalInput").ap()
    m0col_d = nc.dram_tensor("m0col", [128, 1], F32, kind="ExternalInput").ap()

    zw_d = nc.dram_tensor("zw", [128, 2 * nr], F32, kind="ExternalOutput").ap()

    pdt = F32R if st2_f32r else BF16   # paug dtype must match stage2 lhsT

    with tile.TileContext(nc) as tc:
        with (
            tc.tile_pool(name="apool", bufs=NCH) as apool,
            tc.tile_pool(name="cpool", bufs=NCH) as cpool,
            tc.tile_pool(name="rpool", bufs=NCH) as rpool,
            tc.tile_pool(name="r32pool", bufs=NCH if st2_f32r else 1) as r32pool,
            tc.tile_pool(name="paugpool", bufs=NCH) as paugpool,
            tc.tile_pool(name="gapool", bufs=2) as gapool,
            tc.tile_pool(name="gwpool", bufs=2) as gwpool,
            tc.tile_pool(name="Epool", bufs=2) as Epool,
            tc.tile_pool(name="scrpool", bufs=2) as scrpool,
            tc.tile_pool(name="w2pool", bufs=1) as w2pool,
            tc.tile_pool(name="tiny", bufs=6) as tiny,
            tc.tile_pool(name="ps", bufs=8, space="PSUM") as ps,
        ):
            # ---- first stage-1 chunk pair goes first (PE start gate),
            # then the tiny loads, then big chunks interleaved.
            embc_t = [None] * NCH
            at_t = [None] * NCH
            embr_t = [None] * NCH
            embr32_t = [None] * NCH

            t = cpool.tile([128, c], BF16, tag="c", name="embc0")
            nc.sync.dma_start(out=t[:], in_=embc_d[0])
            embc_t[0] = t
            t = apool.tile([128, D], BF16, tag="a", name="at0")
            nc.sync.dma_start(out=t[:], in_=at_d[0])
            at_t[0] = t

            mrows_t = tiny.tile([4, R], F32R, tag="mr")
            nc.sync.dma_start(out=mrows_t[:], in_=mrows_d)
            rhs4_t = tiny.tile([4, c], F32R, tag="r4")
            nc.sync.dma_start(out=rhs4_t[:], in_=rhs4_d)
            ucol_t = tiny.tile([128, NCH], F32, tag="uc")
            nc.sync.dma_start(out=ucol_t[:], in_=ucol_d)
            rrow_t = tiny.tile([1, c], F32, tag="rr")
            nc.sync.dma_start(out=rrow_t[:], in_=rrow_d)
            m0col_t = tiny.tile([128, 1], F32, tag="m0")
            nc.sync.dma_start(out=m0col_t[:], in_=m0col_d)

            for k in range(1, NCH):
                t = cpool.tile([128, c], BF16, tag="c", name=f"embc{k}")
                nc.sync.dma_start(out=t[:], in_=embc_d[k])
                embc_t[k] = t
                t = apool.tile([128, D], BF16, tag="a", name=f"at{k}")
                nc.sync.dma_start(out=t[:], in_=at_d[k])
                at_t[k] = t
                t = rpool.tile([128, R], BF16, tag="r", name=f"embr{k - 1}")
                nc.sync.dma_start(out=t[:], in_=embr_d[k - 1])
                embr_t[k - 1] = t
            t = rpool.tile([128, R], BF16, tag="r", name=f"embr{NCH - 1}")
            nc.sync.dma_start(out=t[:], in_=embr_d[NCH - 1])
            embr_t[NCH - 1] = t
            if st2_f32r:
                for k in range(NCH):
                    t = r32pool.tile([128, R], F32R, tag="r32", name=f"embr32_{k}")
                    nc.sync.dma_start(out=t[:], in_=embr32_d[k])
                    embr32_t[k] = t

            # ---- stage 1: P = at.T @ embc  (db-outer over 8 banks)
            st1 = [ps.tile([128, 512], F32, tag="ps", name=f"st1_{da}")
                   for da in range(NCH)]
            for db in range(NCH):
                for da in range(NCH):
                    nc.tensor.matmul(st1[da][:, 0:c],
                                     at_t[db][:, da * 128:(da + 1) * 128],
                                     embc_t[db][:],
                                     start=(db == 0), stop=(db == NCH - 1))
            # PSUM -> SBUF with the u-column bias, split across ACT/DVE
            paug = []
            for da in range(NCH):
                pt = paugpool.tile([128, c], pdt, tag="paug")
                if da % 2 == 0:
                    nc.scalar.activation(out=pt[:], in_=st1[da][:, 0:c],
                                         func=AFT.Identity,
                                         bias=ucol_t[:, da:da + 1], scale=1.0)
                else:
                    nc.vector.tensor_scalar_add(pt[:], st1[da][:, 0:c],
                                                ucol_t[:, da:da + 1])
                paug.append(pt)

            # ---- W2 = broadcast of r over cols
            W2 = w2pool.tile([128, c], F32, tag="w2")
            nc.gpsimd.partition_broadcast(W2[:], rrow_t[0:1, :], channels=128)

            # ---- gram -> gw = |G| * r_c  (overlaps the paug copies)
            gw_t = []
            for yc in range(nr):
                Gp = ps.tile([128, 512], F32, tag="ps", name=f"G_{yc}")
                for d2 in range(NCH):
                    nc.tensor.matmul(Gp[:, 0:c],
                                     embr_t[d2][:, yc * 128:(yc + 1) * 128],
                                     embc_t[d2][:],
                                     start=(d2 == 0), stop=(d2 == NCH - 1))
                ga = gapool.tile([128, c], F32, tag="ga")
                nc.scalar.activation(out=ga[:], in_=Gp[:, 0:c], func=AFT.Abs,
                                     bias=0.0, scale=1.0)
                gw = gwpool.tile([128, c], F32, tag="gw")
                nc.vector.tensor_mul(gw[:], ga[:], W2[:])
                gw_t.append(gw)

            # ---- stage 2: L = mask + embr.T @ paug; exp/stt with accums
            ztile = tiny.tile([128, nr], F32, tag="z")
            wtile = tiny.tile([128, nr], F32, tag="w")
            lhs_t = embr32_t if st2_f32r else embr_t
            for yc in range(nr):
                Lp = ps.tile([128, 512], F32, tag="ps", name=f"L_{yc}")
                nc.tensor.matmul(Lp[:, 0:c], mrows_t[:, yc * 128:(yc + 1) * 128],
                                 rhs4_t[:], start=True, stop=False)
                for da in range(NCH):
                    nc.tensor.matmul(Lp[:, 0:c],
                                     lhs_t[da][:, yc * 128:(yc + 1) * 128],
                                     paug[da][:], start=False, stop=(da == NCH - 1))
                E = Epool.tile([128, c], F32, tag="E")
                nc.scalar.activation(out=E[:], in_=Lp[:, 0:c], func=AFT.Exp,
                                     bias=m0col_t[:], scale=1.0,
                                     accum_out=ztile[:, yc:yc + 1])
                scr = scrpool.tile([128, c], F32, tag="scr")
                nc.vector.scalar_tensor_tensor(
                    out=scr[:], in0=gw_t[yc][:], scalar=1.0, in1=E[:],
                    op0=ALU.mult, op1=ALU.mult,
                    accum_out=wtile[:, yc:yc + 1])

            nc.sync.dma_start(out=zw_d[:, 0:nr], in_=ztile[:])
            nc.sync.dma_start(out=zw_d[:, nr:2 * nr], in_=wtile[:])

    nc.compile()
    _builds[key] = nc
    return nc


def _pick_pairing(n_rows: np.ndarray, n_cols: np.ndarray):
    """Pair the 16 batches into 8 cores: rows (m1) must fit 2*128 chunks,
    cols (m0) set the free dim; minimize the max col sum."""
    def pairs_from(order):
        return [(int(order[k]), int(order[B - 1 - k])) for k in range(B // 2)]

    best = None
    for key in (-n_cols, -n_rows):
        pr = pairs_from(np.argsort(key, kind="stable"))
        rmax = max(n_rows[a] + n_rows[b] for a, b in pr)
        cmax = max(n_cols[a] + n_cols[b] for a, b in pr)
        cand = (int(np.ceil(max(rmax, 1) / 128)), int(cmax), pr)
        if best is None or (cand[0], cand[1]) < (best[0], best[1]):
            best = cand
    nr, cmax, pr = best
    c = max(256, -(-max(cmax, 1) // 4) * 4)
    return nr, c, pr


def kernel(embeddings, Wq, bq, Wk, bk, attention_masks, token_type_ids):
    global LAST_RESULTS

    emb = np.ascontiguousarray(np.asarray(embeddings, dtype=np.float32))
    Wq = np.asarray(Wq, dtype=np.float32)
    Wk = np.asarray(Wk, dtype=np.float32)
    bq = np.asarray(bq, dtype=np.float32)
    bk = np.asarray(bk, dtype=np.float32)
    am = np.asarray(attention_masks)
    tt = np.asarray(token_type_ids)

    tok = am == 1
    m0 = tok & (tt == 0)   # cols
    m1 = tok & (tt == 1)   # rows
    n_cols = m0.sum(1)
    n_rows = m1.sum(1)

    nr, c, pairing = _pick_pairing(n_rows, n_cols)
    R = nr * 128
    nc = _build(nr, c, ST2_F32R)

    # ---- host constant folding
    Wq64, Wk64 = Wq.astype(np.float64), Wk.astype(np.float64)
    A = (Wq64.T @ Wk64).astype(np.float32)          # [db, da] stage-1 lhsT
    at16 = _bf16(A).reshape(NCH, 128, D)
    u = (Wk64.T @ bq.astype(np.float64)).astype(np.float32)       # P bias
    ucol = np.ascontiguousarray(u.reshape(NCH, 128).T)            # [128, NCH]
    u2 = Wq64.T @ bk.astype(np.float64)             # prow direction
    c0 = float(bq.astype(np.float64) @ bk.astype(np.float64))

    nrm = np.sqrt(np.einsum("bsd,bsd->bs", emb, emb, dtype=np.float64))
    rr_full = (1.0 / np.maximum(nrm, EPS)).astype(np.float32)     # [B, S]

    in_maps = []
    row_meta = []   # per core: (b0, nrow0, b1, nrow1, r_rows[R])
    for (b0, b1) in pairing:
        ridx = [(b, j) for b in (b0, b1) for j in np.nonzero(m1[b])[0]]
        cidx = [(b, j) for b in (b0, b1) for j in np.nonzero(m0[b])[0]]
        nrow0 = int(n_rows[b0])
        ncol0 = int(n_cols[b0])
        nrow = len(ridx)
        ncol = len(cidx)

        er = np.zeros((R, D), np.float32)
        for i, (b, j) in enumerate(ridx):
            er[i] = emb[b, j]
        ec = np.zeros((c, D), np.float32)
        for i, (b, j) in enumerate(cidx):
            ec[i] = emb[b, j]

        embr = _bf16(er.T.reshape(NCH, 128, R))
        embc = _bf16(ec.T.reshape(NCH, 128, c))

        prow = (ec.astype(np.float64) @ u2 + c0).astype(np.float32)
        prow[ncol:] = NEG                       # padded cols masked via row0

        mrows = np.zeros((4, R), np.float32)
        mrows[0, :] = 1.0
        mrows[1, :nrow0] = 1.0                  # b0 rows
        mrows[2, nrow0:nrow] = 1.0              # b1 rows
        mrows[3, nrow:] = 1.0                   # padded rows
        rhs4 = np.zeros((4, c), np.float32)
        rhs4[0] = prow
        rhs4[1, ncol0:ncol] = NEG               # b1 cols, masked for b0 rows
        rhs4[2, :ncol0] = NEG                   # b0 cols, masked for b1 rows
        rhs4[3, :] = NEG                        # all cols, masked for pad rows

        rrow = np.zeros((1, c), np.float32)
        r_cols = np.array([rr_full[b, j] for (b, j) in cidx], np.float32)
        rrow[0, :ncol] = r_cols
        r_rows = np.zeros(R, np.float32)
        r_rows[:nrow] = [rr_full[b, j] for (b, j) in ridx]

        im = {
            "at": at16,
            "embc": embc,
            "embr": embr,
            "mrows": _to_fp32r(mrows),
            "rhs4": _to_fp32r(rhs4),
            "ucol": ucol,
            "rrow": rrow,
            "m0col": np.full((128, 1), -M0, np.float32),
        }
        if ST2_F32R:
            im["embr32"] = _to_fp32r(er.T.reshape(NCH, 128, R))
        in_maps.append(im)
        row_meta.append((b0, nrow0, b1, nrow - nrow0))

    valid = m0.any(axis=1) & m1.any(axis=1)
    for attempt in range(3):
        res = run_bass_kernel_spmd(nc, in_maps, core_ids=list(range(NCORES)),
                                   trace=PROFILE)
        LAST_RESULTS = res
        ok = all(np.isfinite(res.results[i]["zw"]).all() for i in range(NCORES))
        if ok:
            break
        for im in in_maps:    # overflow escape hatch: larger shift, no recompile
            im["m0col"] = im["m0col"] * 4.0

    cs = np.zeros(B, np.float64)
    for i in range(NCORES):
        zw = res.results[i]["zw"].astype(np.float64)      # [128, 2*nr]
        zflat = zw[:, 0:nr].T.ravel()                     # row-major [R]
        wflat = zw[:, nr:2 * nr].T.ravel()
        b0, nrow0, b1, nrow1 = row_meta[i]
        r_rows = np.zeros(R, np.float64)
        ridx = [(b, j) for b in (b0, b1) for j in np.nonzero(m1[b])[0]]
        r_rows[:len(ridx)] = [rr_full[b, j] for (b, j) in ridx]
        wr = wflat * r_rows
        if valid[b0]:
            z = zflat[:nrow0].sum()
            cs[b0] = wr[:nrow0].sum() / (z + 1e-300)
        if valid[b1]:
            z = zflat[nrow0:nrow0 + nrow1].sum()
            cs[b1] = wr[nrow0:nrow0 + nrow1].sum() / (z + 1e-300)
    return cs.astype(np.float32)


# revision 12
# speedup vs baseline: 2.6432x; 1.1267x over previous
"""Trainium2 Bass kernel for nn_CESAR_24309514895978 (ragged_sequence).

Math (per batch b):
  m0 = (am==1)&(tt==0); m1 = (am==1)&(tt==1)
  score[i,j] = |emb_n[i] . emb_n[j]|   (L2-normalized embeddings)
  logits[i,j] = (emb@Wq.T+bq)[i] . (emb@Wk.T+bk)[j]
  cs[b] = sum_{valid ij} softmax_flat(logits | i in m0, j in m1)[i,j] * score[i,j]

Ragged compaction: only ~25% of tokens are in m0 and ~25% in m1, so the
host gathers the valid tokens and the device works on compacted panels:
rows = m1 tokens of 2 batches packed (<=2*128), cols = m0 tokens (free
dim ~260).  Matmul cost scales with the free dim only, so rows use the
partition dim (2 chunks) and cols the free dim.

Constant folding (host): logits = embaug_r @ A_aug @ embaug_c.T with
A_aug = [[Wk.T@Wq, Wk.T@bq], [bk.T@Wq, bq.bk]].  The device gets
  at  = (Wq.T@Wk)[db, da]  (stage-1 lhsT, bf16 to halve its DMA)
  ucol= Wk.T@bq            (bias riding the PSUM->SBUF copy of P)
  prow= emb_c@(Wq.T@bk)+bq.bk  (host-computed rank-1 row, in the mask mm)
Norms r=1/||emb|| are computed on the host; r_c rides a broadcast row,
r_r is applied host-side to the W partials.

Batch identity inside a packed panel is enforced with a K=4 mask matmul
(sum of non-positive rank-1 terms; no large-value cancellation):
  [ones, b0r, b1r, padr] x [prow, -1e30*b1c, -1e30*b0c, -1e30*ones]
No on-device max: exp uses a constant bias -M0 (uploaded, so a retry
with a larger M0 needs no recompile); W/Z ratio cancels the shift.

Device per core: stage1 P = at.T @ embc (64 mm), gram G = embr.T @ embc
(16 mm), stage2 L = embr.T @ paug + mask (18 mm); exp+accum -> Z rows,
stt(gw,E)+accum -> W rows.  Host: segment-sum rows by batch, cs = W/Z.
"""
import numpy as np
import ml_dtypes

import concourse.tile as tile
from concourse import bacc, mybir
from concourse.bass_utils import run_bass_kernel_spmd

B, S, D = 16, 512, 1024
NCORES = 8
NCH = D // 128             # 8 contraction chunks
NEG = np.float32(-1e30)
M0 = 60.0                  # logit shift; exp(L - M0), max logit ~58
EPS = 1e-12

F32 = mybir.dt.float32
F32R = mybir.dt.float32r
BF16 = mybir.dt.bfloat16
AFT = mybir.ActivationFunctionType
ALU = mybir.AluOpType

PROFILE = False            # set True (e.g. from test.py) to capture NTFF profile
LAST_RESULTS = None        # BassKernelResults of the last run (for test.py)

ST2_F32R = True            # stage2 (embr x paug) in f32r instead of bf16

_builds = {}


def _to_fp32r(x: np.ndarray) -> np.ndarray:
    """Round fp32 -> fp32r encoding (RNE to 11 explicit mantissa bits)."""
    u = np.ascontiguousarray(x, dtype=np.float32).view(np.uint32).astype(np.uint64)
    u = (u + 0x7FF + ((u >> 12) & 1)) & np.uint64(0xFFFFF000)
    return u.astype(np.uint32).view(np.float32)


def _bf16(x: np.ndarray) -> np.ndarray:
    return np.ascontiguousarray(np.asarray(x, np.float32)).astype(ml_dtypes.bfloat16)


def _build(nr: int, c: int, st2_f32r: bool):
    key = (nr, c, st2_f32r)
    if key in _builds:
        return _builds[key]

    R = nr * 128
    nc = bacc.Bacc("TRN2", target_bir_lowering=False, debug=False)

    # emb panels are packed chunk-major along the free dim so one DMA moves
    # 4KB+ per partition line (per-partition descriptors stay efficient)
    at_d = nc.dram_tensor("at", [NCH, 128, D], BF16, kind="ExternalInput").ap()
    embc_d = nc.dram_tensor("embc", [128, NCH * c], BF16, kind="ExternalInput").ap()
    embr_d = nc.dram_tensor("embr", [128, NCH * R], BF16, kind="ExternalInput").ap()
    if st2_f32r:
        embr32_d = nc.dram_tensor("embr32", [128, NCH * R], F32R,
                                  kind="ExternalInput").ap()
    mrows_d = nc.dram_tensor("mrows", [4, R], F32R, kind="ExternalInput").ap()
    rhs4_d = nc.dram_tensor("rhs4", [4, c], F32R, kind="ExternalInput").ap()
    ucol_d = nc.dram_tensor("ucol", [128, NCH], F32, kind="ExternalInput").ap()
    rrow_d = nc.dram_tensor("rrow", [1, c], F32, kind="ExternalInput").ap()
    m0col_d = nc.dram_tensor("m0col", [128, 1], F32, kind="ExternalInput").ap()

    zw_d = nc.dram_tensor("zw", [128, 2 * nr], F32, kind="ExternalOutput").ap()

    pdt = F32R if st2_f32r else BF16   # paug dtype must match stage2 lhsT

    with tile.TileContext(nc) as tc:
        with (
            tc.tile_pool(name="apool", bufs=NCH) as apool,
            tc.tile_pool(name="cpool", bufs=NCH) as cpool,
            tc.tile_pool(name="rpool", bufs=NCH) as rpool,
            tc.tile_pool(name="r32pool", bufs=NCH if st2_f32r else 1) as r32pool,
            tc.tile_pool(name="paugpool", bufs=NCH) as paugpool,
            tc.tile_pool(name="gapool", bufs=2) as gapool,
            tc.tile_pool(name="gwpool", bufs=2) as gwpool,
            tc.tile_pool(name="Epool", bufs=2) as Epool,
            tc.tile_pool(name="scrpool", bufs=2) as scrpool,
            tc.tile_pool(name="w2pool", bufs=1) as w2pool,
            tc.tile_pool(name="tiny", bufs=6) as tiny,
            tc.tile_pool(name="ps", bufs=8, space="PSUM") as ps,
        ):
            # ---- DMA order: embc halves + at[0] gate the PE start; at[1..7]
            # keep stage 1 streaming; embr lands by gram time, embr32 by
            # stage 2; tinies ride along early.
            hw = NCH * c // 2
            embc_t = cpool.tile([128, NCH * c], BF16, tag="c", name="embc")
            nc.sync.dma_start(out=embc_t[:, 0:hw], in_=embc_d[:, 0:hw])
            at_t = [None] * NCH
            t = apool.tile([128, D], BF16, tag="a", name="at0")
            nc.sync.dma_start(out=t[:], in_=at_d[0])
            at_t[0] = t
            nc.sync.dma_start(out=embc_t[:, hw:], in_=embc_d[:, hw:])

            mrows_t = tiny.tile([4, R], F32R, tag="mr")
            nc.sync.dma_start(out=mrows_t[:], in_=mrows_d)
            rhs4_t = tiny.tile([4, c], F32R, tag="r4")
            nc.sync.dma_start(out=rhs4_t[:], in_=rhs4_d)
            ucol_t = tiny.tile([128, NCH], F32, tag="uc")
            nc.sync.dma_start(out=ucol_t[:], in_=ucol_d)
            rrow_t = tiny.tile([1, c], F32, tag="rr")
            nc.sync.dma_start(out=rrow_t[:], in_=rrow_d)
            m0col_t = tiny.tile([128, 1], F32, tag="m0")
            nc.sync.dma_start(out=m0col_t[:], in_=m0col_d)

            for k in range(1, NCH):
                t = apool.tile([128, D], BF16, tag="a", name=f"at{k}")
                nc.sync.dma_start(out=t[:], in_=at_d[k])
                at_t[k] = t
            embr_t = rpool.tile([128, NCH * R], BF16, tag="r", name="embr")
            nc.sync.dma_start(out=embr_t[:], in_=embr_d)
            embr32_t = None
            if st2_f32r:
                embr32_t = r32pool.tile([128, NCH * R], F32R, tag="r32",
                                        name="embr32")
                nc.sync.dma_start(out=embr32_t[:], in_=embr32_d)

            # ---- stage 1: P = at.T @ embc  (db-outer over 8 banks)
            st1 = [ps.tile([128, 512], F32, tag="ps", name=f"st1_{da}")
                   for da in range(NCH)]
            for db in range(NCH):
                for da in range(NCH):
                    nc.tensor.matmul(st1[da][:, 0:c],
                                     at_t[db][:, da * 128:(da + 1) * 128],
                                     embc_t[:, db * c:(db + 1) * c],
                                     start=(db == 0), stop=(db == NCH - 1))
            # PSUM -> SBUF with the u-column bias, split across ACT/DVE
            paug = []
            for da in range(NCH):
                pt = paugpool.tile([128, c], pdt, tag="paug")
                if da % 2 == 0:
                    nc.scalar.activation(out=pt[:], in_=st1[da][:, 0:c],
                                         func=AFT.Identity,
                                         bias=ucol_t[:, da:da + 1], scale=1.0)
                else:
                    nc.vector.tensor_scalar_add(pt[:], st1[da][:, 0:c],
                                                ucol_t[:, da:da + 1])
                paug.append(pt)

            # ---- W2 = broadcast of r over cols
            W2 = w2pool.tile([128, c], F32, tag="w2")
            nc.gpsimd.partition_broadcast(W2[:], rrow_t[0:1, :], channels=128)

            # ---- gram -> gw = |G| * r_c  (overlaps the paug copies)
            gw_t = []
            for yc in range(nr):
                Gp = ps.tile([128, 512], F32, tag="ps", name=f"G_{yc}")
                for d2 in range(NCH):
                    nc.tensor.matmul(Gp[:, 0:c],
                                     embr_t[:, d2 * R + yc * 128:
                                            d2 * R + (yc + 1) * 128],
                                     embc_t[:, d2 * c:(d2 + 1) * c],
                                     start=(d2 == 0), stop=(d2 == NCH - 1))
                ga = gapool.tile([128, c], F32, tag="ga")
                nc.scalar.activation(out=ga[:], in_=Gp[:, 0:c], func=AFT.Abs,
                                     bias=0.0, scale=1.0)
                gw = gwpool.tile([128, c], F32, tag="gw")
                nc.vector.tensor_mul(gw[:], ga[:], W2[:])
                gw_t.append(gw)

            # ---- stage 2: L = mask + embr.T @ paug; exp/stt with accums
            ztile = tiny.tile([128, nr], F32, tag="z")
            wtile = tiny.tile([128, nr], F32, tag="w")
            lhs_t = embr32_t if st2_f32r else embr_t
            for yc in range(nr):
                Lp = ps.tile([128, 512], F32, tag="ps", name=f"L_{yc}")
                nc.tensor.matmul(Lp[:, 0:c], mrows_t[:, yc * 128:(yc + 1) * 128],
                                 rhs4_t[:], start=True, stop=False)
                for da in range(NCH):
                    nc.tensor.matmul(Lp[:, 0:c],
                                     lhs_t[:, da * R + yc * 128:
                                           da * R + (yc + 1) * 128],
                                     paug[da][:], start=False, stop=(da == NCH - 1))
                E = Epool.tile([128, c], F32, tag="E")
                nc.scalar.activation(out=E[:], in_=Lp[:, 0:c], func=AFT.Exp,
                                     bias=m0col_t[:], scale=1.0,
                                     accum_out=ztile[:, yc:yc + 1])
                scr = scrpool.tile([128, c], F32, tag="scr")
                nc.vector.scalar_tensor_tensor(
                    out=scr[:], in0=gw_t[yc][:], scalar=1.0, in1=E[:],
                    op0=ALU.mult, op1=ALU.mult,
                    accum_out=wtile[:, yc:yc + 1])

            nc.sync.dma_start(out=zw_d[:, 0:nr], in_=ztile[:])
            nc.sync.dma_start(out=zw_d[:, nr:2 * nr], in_=wtile[:])

    nc.compile()
    _builds[key] = nc
    return nc


def _pick_pairing(n_rows: np.ndarray, n_cols: np.ndarray):
    """Pair the 16 batches into 8 cores: rows (m1) must fit 2*128 chunks,
    cols (m0) set the free dim; minimize the max col sum."""
    def pairs_from(order):
        return [(int(order[k]), int(order[B - 1 - k])) for k in range(B // 2)]

    best = None
    for key in (-n_cols, -n_rows):
        pr = pairs_from(np.argsort(key, kind="stable"))
        rmax = max(n_rows[a] + n_rows[b] for a, b in pr)
        cmax = max(n_cols[a] + n_cols[b] for a, b in pr)
        cand = (int(np.ceil(max(rmax, 1) / 128)), int(cmax), pr)
        if best is None or (cand[0], cand[1]) < (best[0], best[1]):
            best = cand
    nr, cmax, pr = best
    c = max(256, -(-max(cmax, 1) // 4) * 4)
    return nr, c, pr


def kernel(embeddings, Wq, bq, Wk, bk, attention_masks, token_type_ids):
    global LAST_RESULTS

    emb = np.ascontiguousarray(np.asarray(embeddings, dtype=np.float32))
    Wq = np.asarray(Wq, dtype=np.float32)
    Wk = np.asarray(Wk, dtype=np.float32)
    bq = np.asarray(bq, dtype=np.float32)
    bk = np.asarray(bk, dtype=np.float32)
    am = np.asarray(attention_masks)
    tt = np.asarray(token_type_ids)

    tok = am == 1
    m0 = tok & (tt == 0)   # cols
    m1 = tok & (tt == 1)   # rows
    n_cols = m0.sum(1)
    n_rows = m1.sum(1)

    nr, c, pairing = _pick_pairing(n_rows, n_cols)
    R = nr * 128
    nc = _build(nr, c, ST2_F32R)

    # ---- host constant folding
    Wq64, Wk64 = Wq.astype(np.float64), Wk.astype(np.float64)
    A = (Wq64.T @ Wk64).astype(np.float32)          # [db, da] stage-1 lhsT
    at16 = _bf16(A).reshape(NCH, 128, D)
    u = (Wk64.T @ bq.astype(np.float64)).astype(np.float32)       # P bias
    ucol = np.ascontiguousarray(u.reshape(NCH, 128).T)            # [128, NCH]
    u2 = Wq64.T @ bk.astype(np.float64)             # prow direction
    c0 = float(bq.astype(np.float64) @ bk.astype(np.float64))

    nrm = np.sqrt(np.einsum("bsd,bsd->bs", emb, emb, dtype=np.float64))
    rr_full = (1.0 / np.maximum(nrm, EPS)).astype(np.float32)     # [B, S]

    in_maps = []
    row_meta = []   # per core: (b0, nrow0, b1, nrow1, r_rows[R])
    for (b0, b1) in pairing:
        ridx = [(b, j) for b in (b0, b1) for j in np.nonzero(m1[b])[0]]
        cidx = [(b, j) for b in (b0, b1) for j in np.nonzero(m0[b])[0]]
        nrow0 = int(n_rows[b0])
        ncol0 = int(n_cols[b0])
        nrow = len(ridx)
        ncol = len(cidx)

        er = np.zeros((R, D), np.float32)
        for i, (b, j) in enumerate(ridx):
            er[i] = emb[b, j]
        ec = np.zeros((c, D), np.float32)
        for i, (b, j) in enumerate(cidx):
            ec[i] = emb[b, j]

        # pack [tok, D] -> [128, NCH*n]: line p holds chunk-major columns,
        # chunk k at cols [k*n, (k+1)*n), partition p <-> d = k*128+p
        erw = er.T.reshape(NCH, 128, R).transpose(1, 0, 2).reshape(128, NCH * R)
        ecw = ec.T.reshape(NCH, 128, c).transpose(1, 0, 2).reshape(128, NCH * c)
        embr = _bf16(erw)
        embc = _bf16(ecw)

        prow = (ec.astype(np.float64) @ u2 + c0).astype(np.float32)
        prow[ncol:] = NEG                       # padded cols masked via row0

        mrows = np.zeros((4, R), np.float32)
        mrows[0, :] = 1.0
        mrows[1, :nrow0] = 1.0                  # b0 rows
        mrows[2, nrow0:nrow] = 1.0              # b1 rows
        mrows[3, nrow:] = 1.0                   # padded rows
        rhs4 = np.zeros((4, c), np.float32)
        rhs4[0] = prow
        rhs4[1, ncol0:ncol] = NEG               # b1 cols, masked for b0 rows
        rhs4[2, :ncol0] = NEG                   # b0 cols, masked for b1 rows
        rhs4[3, :] = NEG                        # all cols, masked for pad rows

        rrow = np.zeros((1, c), np.float32)
        r_cols = np.array([rr_full[b, j] for (b, j) in cidx], np.float32)
        rrow[0, :ncol] = r_cols
        r_rows = np.zeros(R, np.float32)
        r_rows[:nrow] = [rr_full[b, j] for (b, j) in ridx]

        im = {
            "at": at16,
            "embc": embc,
            "embr": embr,
            "mrows": _to_fp32r(mrows),
            "rhs4": _to_fp32r(rhs4),
            "ucol": ucol,
            "rrow": rrow,
            "m0col": np.full((128, 1), -M0, np.float32),
        }
        if ST2_F32R:
            im["embr32"] = _to_fp32r(erw)
        in_maps.append(im)
        row_meta.append((b0, nrow0, b1, nrow - nrow0))

    valid = m0.any(axis=1) & m1.any(axis=1)
    for attempt in range(3):
        res = run_bass_kernel_spmd(nc, in_maps, core_ids=list(range(NCORES)),
                                   trace=PROFILE)
        LAST_RESULTS = res
        ok = all(np.isfinite(res.results[i]["zw"]).all() for i in range(NCORES))
        if ok:
            break
        for im in in_maps:    # overflow escape hatch: larger shift, no recompile
            im["m0col"] = im["m0col"] * 4.0

    cs = np.zeros(B, np.float64)
    for i in range(NCORES):
        zw = res.results[i]["zw"].astype(np.float64)      # [128, 2*nr]
        zflat = zw[:, 0:nr].T.ravel()                     # row-major [R]
        wflat = zw[:, nr:2 * nr].T.ravel()
        b0, nrow0, b1, nrow1 = row_meta[i]
        r_rows = np.zeros(R, np.float64)
        ridx = [(b, j) for b in (b0, b1) for j in np.nonzero(m1[b])[0]]
        r_rows[:len(ridx)] = [rr_full[b, j] for (b, j) in ridx]
        wr = wflat * r_rows
        if valid[b0]:
            z = zflat[:nrow0].sum()
            cs[b0] = wr[:nrow0].sum() / (z + 1e-300)
        if valid[b1]:
            z = zflat[nrow0:nrow0 + nrow1].sum()
            cs[b1] = wr[nrow0:nrow0 + nrow1].sum() / (z + 1e-300)
    return cs.astype(np.float32)
